# revision 7
# baseline (speedup 1.0000x reference)
"""Trainium2 Bass kernel for nn_Attention_83141976916236.

Reference computation (B=2, N=2048, C=512, H=8, D=64):
    qkv = x @ qkv_w                       -> split to q, k, v per head
    att_h = softmax(q_h k_h^T / sqrt(D)) v_h        (per batch b, head h)
    out  = reshape_no_transpose(att) @ proj_w + proj_b

Key structural fact: the reference reshapes (B,H,N,D) -> (B,N,C) WITHOUT
transposing, so output row n' = h*256 + n//8 with channel c' = (n%8)*64 + d.
Every output row therefore depends on exactly ONE head: with heads sharded
across cores, each core produces a disjoint slice of output rows and the
host-side unshard is a pure concatenation (no cross-core reduction).

Sharding (8 cores): core c handles batch b = c//4 and heads (2p, 2p+1) where
p = c%4. Each core computes its 2 heads' q/k/v projections, flash-style
attention (scores kept transposed [j,i] so softmax sums come free via an
appended ones-column in the AV matmul), and the output projection for its
512 output rows. All matmuls run in fp32r (full PE rate, ~1e-4 rel err).

Host-side prep per core: x[b] transposed to channel-major (the PE contracts
over the partition axis, so both matmul operands need C on partitions),
qkv_w column slice for its heads, proj_w rearranged for the scrambled-row
projection. Host-side unshard: row-slice concatenation + bias add.
"""

import numpy as np
from contextlib import ExitStack

import concourse.tile as tile
from concourse import bacc, mybir
from concourse.bass_utils import run_bass_kernel_spmd
from concourse.masks import make_identity

B, N, C, H = 2, 2048, 512, 8
D = C // H            # 64
SCALE = D ** -0.5
N_CORES = 8
F32 = mybir.dt.float32
F32R = mybir.dt.float32r
EXP = mybir.ActivationFunctionType.Exp

_programs = {}


def build_program(reps: int = 1, debug: bool = False):
    """Build + compile the SPMD single-core program.

    reps > 1 wraps the whole body in a hardware loop (used only for timing
    calibration). debug=True adds DRAM dumps of intermediates.
    """
    nc = bacc.Bacc("TRN2", target_bir_lowering=False, debug=False,
                   num_devices=N_CORES)
    xt = nc.dram_tensor("xt", [C, N], F32R, kind="ExternalInput").ap()
    wqkv = nc.dram_tensor("wqkv", [C, 384], F32R, kind="ExternalInput").ap()
    wp = nc.dram_tensor("wp", [128, 8, C], F32R, kind="ExternalInput").ap()
    part = nc.dram_tensor("part", [512, C], F32, kind="ExternalOutput").ap()
    dbg = {}
    if debug:
        for name, shape in (("d_qT", [128, N]), ("d_kT", [128, N]),
                            ("d_vext", [128, 16 * 130]), ("d_attn", [128, N])):
            dbg[name] = nc.dram_tensor(name, shape, F32, kind="ExternalOutput").ap()

    with tile.TileContext(nc) as tc, ExitStack() as ctx:
        ctx.enter_context(nc.allow_low_precision(reason="fp32r attention kernel"))
        consts = ctx.enter_context(tc.tile_pool(name="consts", bufs=1))
        bigs = ctx.enter_context(tc.tile_pool(name="bigs", bufs=1))
        probs_pool = ctx.enter_context(tc.tile_pool(name="probs", bufs=3))
        small = ctx.enter_context(tc.tile_pool(name="small", bufs=2))
        outp = ctx.enter_context(tc.tile_pool(name="outp", bufs=2))

        # memset/affine_select can't emit float32r directly; build f32 then
        # cast via DVE copy (a verifier-approved fp32r rounding producer).
        ident_f = consts.tile([128, 128], F32)
        make_identity(nc, ident_f[:])
        ident = consts.tile([128, 128], F32R)
        nc.vector.tensor_copy(out=ident[:], in_=ident_f[:])
        ones_f = consts.tile([128, 128], F32)
        nc.vector.memset(ones_f[:], 1.0)
        ones = consts.tile([1, 128], F32R)
        nc.vector.tensor_copy(out=ones[:], in_=ones_f[0:1, :])
        ones_wide = consts.tile([128, 32], F32R)
        nc.vector.tensor_copy(out=ones_wide[:], in_=ones_f[:, 0:32])

        def body():
            # ---- loads -------------------------------------------------
            xt_sb = bigs.tile([128, 4, N], F32R, tag="xt")
            nc.sync.dma_start(out=xt_sb[:], in_=xt.rearrange("(k p) n -> p k n", p=128))
            wqkv_sb = bigs.tile([128, 4, 384], F32R, tag="wqkv")
            nc.sync.dma_start(out=wqkv_sb[:], in_=wqkv.rearrange("(k p) f -> p k f", p=128))
            wp_sb = bigs.tile([128, 8, C], F32R, tag="wp")
            nc.sync.dma_start(out=wp_sb[:], in_=wp)

            qT = bigs.tile([128, N], F32R, tag="qT")
            kT = bigs.tile([128, N], F32R, tag="kT")
            vT = bigs.tile([128, N], F32R, tag="vT")
            attn = bigs.tile([128, N], F32R, tag="attn")
            # v in row-major [j, 64+ones | 64+ones] blocks; ones col feeds the
            # softmax-denominator row of the AV matmul.
            vext = bigs.tile([128, 16, 130], F32R, tag="vext")
            vext_cols = vext[:].rearrange("p a (b c) -> p a b c", b=2)
            nc.vector.tensor_copy(
                out=vext_cols[:, :, :, 64],
                in_=ones_wide[:].rearrange("p (a b) -> p a b", a=16))

            # ---- qkv projection: qkvT[f, n] = wqkv.T @ xT ---------------
            with tc.tile_pool(name="ps_qkv", bufs=3, space="PSUM") as ps_qkv, \
                 tc.tile_pool(name="ps_vt", bufs=2, space="PSUM") as ps_vt:
                dests = (qT, kT, vT)
                for f in range(3):
                    for nb in range(4):
                        ps = ps_qkv.tile([128, 512], F32, tag="qkv")
                        for k in range(4):
                            nc.tensor.matmul(
                                ps[:], wqkv_sb[:, k, f * 128:(f + 1) * 128],
                                xt_sb[:, k, nb * 512:(nb + 1) * 512],
                                start=(k == 0), stop=(k == 3))
                        nc.vector.tensor_copy(
                            out=dests[f][:, nb * 512:(nb + 1) * 512], in_=ps[:])
                # transpose v to row-major via PE
                for jb in range(16):
                    pst = ps_vt.tile([128, 128], F32R, tag="vt")
                    nc.tensor.transpose(pst[:], vT[:, jb * 128:(jb + 1) * 128], ident[:])
                    nc.vector.tensor_copy(out=vext[:, jb, 0:64], in_=pst[:, 0:64])
                    nc.vector.tensor_copy(out=vext[:, jb, 65:129], in_=pst[:, 64:128])

            # ---- attention + projection --------------------------------
            with tc.tile_pool(name="ps_scr", bufs=2, space="PSUM") as ps_scr, \
                 tc.tile_pool(name="ps_av", bufs=1, space="PSUM") as ps_av, \
                 tc.tile_pool(name="ps_pj", bufs=2, space="PSUM") as ps_pj:
                for h in range(2):
                    hp = slice(64 * h, 64 * h + 64)
                    for ihalf in range(2):
                        i0 = ihalf * 1024
                        av = ps_av.tile([128, 1024], F32, tag="av")
                        for jb in range(16):
                            # scoresT[j, i] for 128 j's x 1024 i's
                            scr = ps_scr.tile([128, 1024], F32, tag="scr")
                            for half in range(2):
                                nc.tensor.matmul(
                                    scr[:, half * 512:(half + 1) * 512],
                                    kT[hp, jb * 128:(jb + 1) * 128],
                                    qT[hp, i0 + half * 512:i0 + (half + 1) * 512],
                                    start=True, stop=True)
                            pr = probs_pool.tile([128, 1024], F32R, tag="pr")
                            nc.scalar.activation(out=pr[:], in_=scr[:], func=EXP,
                                                 scale=SCALE)
                            for half in range(2):
                                nc.tensor.matmul(
                                    av[0:65, half * 512:(half + 1) * 512],
                                    vext[:, jb, 65 * h:65 * h + 65],
                                    pr[:, half * 512:(half + 1) * 512],
                                    start=(jb == 0), stop=(jb == 15))
                        # normalize: rows 0-63 of av are sum_j p*v, row 64 is sum_j p
                        rc = small.tile([1, 1024], F32R, tag="rc")
                        nc.vector.reciprocal(rc[:], av[64:65, :])
                        bc = ps_scr.tile([128, 1024], F32, tag="scr")
                        for half in range(2):
                            nc.tensor.matmul(
                                bc[0:64, half * 512:(half + 1) * 512],
                                ones[0:1, 0:64],
                                rc[0:1, half * 512:(half + 1) * 512],
                                start=True, stop=True)
                        bcs = small.tile([64, 1024], F32R, tag="bcs")
                        nc.vector.tensor_copy(out=bcs[:], in_=bc[0:64, :])
                        nc.vector.tensor_mul(attn[hp, i0:i0 + 1024], av[0:64, :], bcs[:])
                    # projection for head h: out rows n'=h*256+m, contraction
                    # c'=(g,d) with att value at attn[64h+d, 8m+g]
                    attn_h = attn[hp, :].rearrange("p (mb m g) -> p mb m g", mb=2, g=8)
                    for mb in range(2):
                        pp = ps_pj.tile([128, 512], F32, tag="pj")
                        for g in range(8):
                            nc.tensor.matmul(pp[:], attn_h[:, mb, :, g],
                                             wp_sb[hp, g, :],
                                             start=(g == 0), stop=(g == 7))
                        ob = outp.tile([128, 512], F32, tag="ob")
                        nc.vector.tensor_copy(out=ob[:], in_=pp[:])
                        nc.sync.dma_start(
                            out=part.rearrange("(r p) c -> r p c", p=128)[2 * h + mb],
                            in_=ob[:])
            if debug:
                for name, t in (("d_qT", qT), ("d_kT", kT), ("d_attn", attn)):
                    sb = outp.tile([128, N], F32, tag="dbg")
                    nc.vector.tensor_copy(out=sb[:], in_=t[:])
                    nc.sync.dma_start(out=dbg[name], in_=sb[:])
                sb = outp.tile([128, 16 * 130], F32, tag="dbg")
                nc.vector.tensor_copy(out=sb[:], in_=vext[:].rearrange("p a b -> p (a b)"))
                nc.sync.dma_start(out=dbg["d_vext"], in_=sb[:])

        if reps == 1:
            body()
        else:
            with tc.For_i(0, reps, 1):
                body()

    nc.compile()
    return nc


def _get_program(reps: int = 1, debug: bool = False):
    key = (reps, debug)
    if key not in _programs:
        _programs[key] = build_program(reps, debug)
    return _programs[key]


def _in_maps(x, qkv_w, proj_w):
    wp_arr = np.ascontiguousarray(
        np.tile(proj_w.reshape(8, 64, C).transpose(1, 0, 2), (2, 1, 1)))
    maps = []
    for c in range(N_CORES):
        b, p = divmod(c, 4)
        xt = np.ascontiguousarray(x[b].T)
        wqkv = np.ascontiguousarray(np.concatenate(
            [qkv_w[:, t * C + p * 128: t * C + p * 128 + 128] for t in range(3)],
            axis=1))
        maps.append({"xt": xt, "wqkv": wqkv, "wp": wp_arr})
    return maps


def kernel(**inputs) -> np.ndarray:
    x = np.asarray(inputs["x"], np.float32)
    qkv_w = np.asarray(inputs["qkv_w"], np.float32)
    proj_w = np.asarray(inputs["proj_w"], np.float32)
    proj_b = np.asarray(inputs["proj_b"], np.float32)

    nc = _get_program()
    res = run_bass_kernel_spmd(nc, _in_maps(x, qkv_w, proj_w),
                               core_ids=list(range(N_CORES)))
    out = np.empty((B, N, C), np.float32)
    for c in range(N_CORES):
        b, p = divmod(c, 4)
        out[b, p * 512:(p + 1) * 512, :] = res.results[c]["part"]
    out += proj_b
    return out


# revision 24
# speedup vs baseline: 1.8021x; 1.8021x over previous
"""Trainium2 Bass kernel for nn_Attention_83141976916236.

Reference computation (B=2, N=2048, C=512, H=8, D=64):
    qkv = x @ qkv_w                       -> split to q, k, v per head
    att_h = softmax(q_h k_h^T / sqrt(D)) v_h        (per batch b, head h)
    out  = reshape_no_transpose(att) @ proj_w + proj_b

Key structural fact: the reference reshapes (B,H,N,D) -> (B,N,C) WITHOUT
transposing, so output row n' = h*256 + n//8 with channel c' = (n%8)*64 + d.
Every output row therefore depends on exactly ONE head: with heads sharded
across cores, each core produces a disjoint slice of output rows and the
host-side unshard is a pure concatenation (no cross-core reduction).

Sharding (8 cores): core c handles batch b = c//4 and heads (2p, 2p+1) where
p = c%4. Each core computes its 2 heads' q/k/v projections, flash-style
attention (scores kept transposed [j,i] so softmax sums come free via an
appended ones-column in the AV matmul), and the output projection for its
512 output rows. QKV/projection matmuls run in fp32r (~1e-4 rel err);
scores/AV run in fp16 with one explicit ldweights shared per matmul pair
(the self-loading weight path costs ~2x per matmul on HW). Attention
emission is software-pipelined one group ahead so the PE computes
scores(g+1) while ACT evaluates exp(g).

Host-side prep per core: x[b] transposed to channel-major (the PE contracts
over the partition axis, so both matmul operands need C on partitions),
qkv_w column slice for its heads, proj_w rearranged for the scrambled-row
projection. Host-side unshard: row-slice concatenation + bias add.
"""

import numpy as np
from contextlib import ExitStack

import concourse.tile as tile
from concourse import bacc, mybir
from concourse.bass_utils import run_bass_kernel_spmd
from concourse.masks import make_identity

B, N, C, H = 2, 2048, 512, 8
D = C // H            # 64
SCALE = D ** -0.5
N_CORES = 8
F32 = mybir.dt.float32
F32R = mybir.dt.float32r
FP16 = mybir.dt.float16
EXP = mybir.ActivationFunctionType.Exp

_programs = {}


def build_program(reps: int = 1, debug: bool = False, n_jb: int = 16,
                  do_attn: bool = True, do_proj: bool = True,
                  do_qkv: bool = True, exp_half: bool = False,
                  unroll: int = 1, loop_kw: dict | None = None):
    """Build + compile the SPMD single-core program.

    reps > 1 wraps the whole body in a hardware loop (used only for timing
    calibration). debug=True adds DRAM dumps of intermediates. The n_jb /
    do_* knobs build timing-experiment variants (numerically wrong).
    """
    nc = bacc.Bacc("TRN2", target_bir_lowering=False, debug=False,
                   num_devices=N_CORES)
    xt = nc.dram_tensor("xt", [C, N], F32R, kind="ExternalInput").ap()
    wqkv = nc.dram_tensor("wqkv", [C, 384], F32R, kind="ExternalInput").ap()
    wp = nc.dram_tensor("wp", [128, 8, C], F32R, kind="ExternalInput").ap()
    part = nc.dram_tensor("part", [512, C], F32, kind="ExternalOutput").ap()
    dbg = {}
    if debug:
        for name, shape in (("d_qT", [128, N]), ("d_kT", [128, N]),
                            ("d_vext", [128, 16 * 130]), ("d_attn", [128, N])):
            dbg[name] = nc.dram_tensor(name, shape, F32, kind="ExternalOutput").ap()

    with tile.TileContext(nc) as tc, ExitStack() as ctx:
        ctx.enter_context(nc.allow_low_precision(reason="fp32r attention kernel"))
        consts = ctx.enter_context(tc.tile_pool(name="consts", bufs=1))
        bigs = ctx.enter_context(tc.tile_pool(name="bigs", bufs=1))
        probs_pool = ctx.enter_context(tc.tile_pool(name="probs", bufs=4))
        small = ctx.enter_context(tc.tile_pool(name="small", bufs=2))
        outp = ctx.enter_context(tc.tile_pool(name="outp", bufs=2))

        # memset/affine_select can't emit float32r directly; build f32 then
        # cast via DVE copy (a verifier-approved fp32r rounding producer).
        ident_f = consts.tile([128, 128], F32)
        make_identity(nc, ident_f[:])
        ident = consts.tile([128, 128], F32R)
        nc.vector.tensor_copy(out=ident[:], in_=ident_f[:])
        ones_f = consts.tile([128, 128], F32)
        nc.vector.memset(ones_f[:], 1.0)
        ones = consts.tile([1, 128], F32R)
        nc.vector.tensor_copy(out=ones[:], in_=ones_f[0:1, :])
        ones_wide = consts.tile([128, 32], FP16)
        nc.vector.tensor_copy(out=ones_wide[:], in_=ones_f[:, 0:32])

        def body():
            # ---- loads -------------------------------------------------
            # weights first (small), then x in 4 n-chunks so the first QKV
            # matmuls start ~6us in instead of waiting for the full 4MB.
            wqkv_sb = bigs.tile([128, 4, 384], F32R, tag="wqkv")
            nc.sync.dma_start(out=wqkv_sb[:], in_=wqkv.rearrange("(k p) f -> p k f", p=128))
            xt_sb = bigs.tile([128, 4, 4, 512], F32R, tag="xt")
            xt_v = xt.rearrange("(k p) (nb n) -> p k nb n", p=128, nb=4)
            for nb in range(4):
                nc.sync.dma_start(out=xt_sb[:, :, nb, :], in_=xt_v[:, :, nb, :])
            wp_sb = bigs.tile([128, 8, C], F32R, tag="wp")
            nc.sync.dma_start(out=wp_sb[:], in_=wp)

            qT = bigs.tile([128, N], FP16, tag="qT")
            kT = bigs.tile([128, N], FP16, tag="kT")
            vT = bigs.tile([128, N], F32R, tag="vT")
            attn = bigs.tile([128, N], F32R, tag="attn")
            # v in row-major [j, 64+ones | 64+ones] blocks; ones col feeds the
            # softmax-denominator row of the AV matmul.
            vext = bigs.tile([128, 16, 130], FP16, tag="vext")
            vext_cols = vext[:].rearrange("p a (b c) -> p a b c", b=2)
            nc.vector.tensor_copy(
                out=vext_cols[:, :, :, 64],
                in_=ones_wide[:].rearrange("p (a b) -> p a b", a=16))

            # PSUM: qkv/vt 2 banks + scr/bc/pj 4 banks + av 2 banks = 8.
            with tc.tile_pool(name="ps_qkv", bufs=2, space="PSUM") as ps_qkv, \
                 tc.tile_pool(name="ps_scr", bufs=2, space="PSUM") as ps_scr, \
                 tc.tile_pool(name="ps_av", bufs=1, space="PSUM") as ps_av:
                dests = (qT, kT, vT)

                def qkv_chunk(nb):
                    for f in range(3 if do_qkv else 0):
                        ps = ps_qkv.tile([128, 512], F32, tag="qkv")
                        for k in range(4):
                            nc.tensor.matmul(
                                ps[:], wqkv_sb[:, k, f * 128:(f + 1) * 128],
                                xt_sb[:, k, nb, :],
                                start=(k == 0), stop=(k == 3))
                        nc.vector.tensor_copy(
                            out=dests[f][:, nb * 512:(nb + 1) * 512], in_=ps[:])
                    # transpose this n-chunk of v to row-major via PE
                    for jb in range(4 * nb, 4 * nb + 4):
                        pst = ps_qkv.tile([128, 128], F32R, tag="qkv")
                        nc.tensor.transpose(pst[:], vT[:, jb * 128:(jb + 1) * 128], ident[:])
                        nc.vector.tensor_copy(out=vext[:, jb, 0:64], in_=pst[:, 0:64])
                        nc.vector.tensor_copy(out=vext[:, jb, 65:129], in_=pst[:, 64:128])

                def scores_g(h, ihalf, jb):
                    # scoresT[j, i] for 128 j's x 1024 i's; one explicit
                    # weight load shared by both i-half matmuls (the
                    # self-loading path costs ~2x per matmul)
                    hp = slice(64 * h, 64 * h + 64)
                    i0 = ihalf * 1024
                    tp = (64 * h, 0)
                    scr = ps_scr.tile([128, 1024], F32, tag="scr")
                    kblk = kT[hp, jb * 128:(jb + 1) * 128]
                    nc.tensor.ldweights(weights=kblk, tile_position=tp)
                    for half in range(2):
                        mm = nc.tensor.matmul(
                            scr[:, half * 512:(half + 1) * 512],
                            kblk,
                            qT[hp, i0 + half * 512:i0 + (half + 1) * 512],
                            start=True, stop=True, tile_position=tp)
                        mm.ins.ldweights = False
                    return scr

                def exp_g(scr):
                    pr = probs_pool.tile([128, 1024], FP16, tag="pr")
                    if exp_half:
                        # timing experiment: half the ACT work, same PE work
                        nc.scalar.activation(out=pr[:, 0:512], in_=scr[:, 0:512],
                                             func=EXP, scale=SCALE)
                    else:
                        nc.scalar.activation(out=pr[:], in_=scr[:], func=EXP,
                                             scale=SCALE)
                    return pr

                def av_g(h, av, pr, jb):
                    vblk = vext[:, jb, 65 * h:65 * h + 65]
                    nc.tensor.ldweights(weights=vblk)
                    for half in range(2):
                        mm = nc.tensor.matmul(
                            av[0:65, half * 512:(half + 1) * 512],
                            vblk,
                            pr[:, 0:512] if exp_half else
                            pr[:, half * 512:(half + 1) * 512],
                            start=(jb == 0), stop=(jb == n_jb - 1))
                        mm.ins.ldweights = False

                def att_norm(h, ihalf, av):
                    # rows 0-63 of av are sum_j p*v, row 64 is sum_j p
                    hp = slice(64 * h, 64 * h + 64)
                    i0 = ihalf * 1024
                    rc = small.tile([1, 1024], F32R, tag="rc")
                    nc.vector.reciprocal(rc[:], av[64:65, :])
                    bc = ps_scr.tile([128, 1024], F32, tag="scr")
                    for half in range(2):
                        nc.tensor.matmul(
                            bc[0:64, half * 512:(half + 1) * 512],
                            ones[0:1, 0:64],
                            rc[0:1, half * 512:(half + 1) * 512],
                            start=True, stop=True)
                    bcs = small.tile([64, 1024], F32R, tag="bcs")
                    nc.vector.tensor_copy(out=bcs[:], in_=bc[0:64, :])
                    nc.vector.tensor_mul(attn[hp, i0:i0 + 1024], av[0:64, :], bcs[:])

                def proj(h):
                    # projection for head h: out rows n'=h*256+m, contraction
                    # c'=(g,d) with att value at attn[64h+d, 8m+g]
                    hp = slice(64 * h, 64 * h + 64)
                    attn_h = attn[hp, :].rearrange("p (mb m g) -> p mb m g", mb=2, g=8)
                    for mb in range(2):
                        pp = ps_scr.tile([128, 512], F32, tag="scr")
                        for g in range(8 if do_proj else 1):
                            nc.tensor.matmul(pp[:], attn_h[:, mb, :, g],
                                             wp_sb[hp, g, :],
                                             start=(g == 0), stop=(g == (7 if do_proj else 0)))
                        ob = outp.tile([128, 512], F32, tag="ob")
                        nc.vector.tensor_copy(out=ob[:], in_=pp[:])
                        nc.sync.dma_start(
                            out=part.rearrange("(r p) c -> r p c", p=128)[2 * h + mb],
                            in_=ob[:])

                # Software-pipelined emission: scores of group g+1 are
                # emitted BEFORE av of group g so the static schedule lets
                # the PE run ahead while ACT evaluates exp(g); the back half
                # of QKV is spread into the early attention stream.
                qkv_chunk(0)
                qkv_chunk(1)
                if do_attn:
                    groups = [(h, ihalf, jb)
                              for h in range(2) for ihalf in range(2)
                              for jb in range(n_jb)]
                    avs = {}
                    scr = scores_g(*groups[0])
                    for idx, (h, ihalf, jb) in enumerate(groups):
                        if jb == 0:
                            av_t = ps_av.tile([128, 1024], F32, tag="av",
                                              name=f"av_{h}_{ihalf}")
                            avs[(h, ihalf)] = av_t
                        pr = exp_g(scr)
                        if idx + 1 < len(groups):
                            scr = scores_g(*groups[idx + 1])
                        av_g(h, avs[(h, ihalf)], pr, jb)
                        if idx == min(3, n_jb - 1):
                            qkv_chunk(2)
                        if idx == min(7, n_jb - 1) + (0 if n_jb > 7 else 1):
                            qkv_chunk(3)
                        if jb == n_jb - 1:
                            att_norm(h, ihalf, avs.pop((h, ihalf)))
                            if (h, ihalf) == (1, 0):
                                proj(0)
                            elif (h, ihalf) == (1, 1):
                                proj(1)
                else:
                    qkv_chunk(2)
                    qkv_chunk(3)
                    proj(0)
                    proj(1)
            if debug:
                for name, t in (("d_qT", qT), ("d_kT", kT), ("d_attn", attn)):
                    sb = outp.tile([128, N], F32, tag="dbg")
                    nc.vector.tensor_copy(out=sb[:], in_=t[:])
                    nc.sync.dma_start(out=dbg[name], in_=sb[:])
                sb = outp.tile([128, 16 * 130], F32, tag="dbg")
                nc.vector.tensor_copy(out=sb[:], in_=vext[:].rearrange("p a b -> p (a b)"))
                nc.sync.dma_start(out=dbg["d_vext"], in_=sb[:])

        if reps == 1:
            for _ in range(unroll):
                body()
        else:
            assert reps % unroll == 0
            with tc.For_i(0, reps // unroll, 1, **(loop_kw or {})):
                for _ in range(unroll):
                    body()

    nc.compile()
    return nc


def _get_program(reps: int = 1, debug: bool = False, **kw):
    key = (reps, debug, repr(sorted(kw.items())))
    if key not in _programs:
        _programs[key] = build_program(reps, debug, **kw)
    return _programs[key]


def _in_maps(x, qkv_w, proj_w):
    wp_arr = np.ascontiguousarray(
        np.tile(proj_w.reshape(8, 64, C).transpose(1, 0, 2), (2, 1, 1)))
    maps = []
    for c in range(N_CORES):
        b, p = divmod(c, 4)
        xt = np.ascontiguousarray(x[b].T)
        wqkv = np.ascontiguousarray(np.concatenate(
            [qkv_w[:, t * C + p * 128: t * C + p * 128 + 128] for t in range(3)],
            axis=1))
        maps.append({"xt": xt, "wqkv": wqkv, "wp": wp_arr})
    return maps


def kernel(**inputs) -> np.ndarray:
    x = np.asarray(inputs["x"], np.float32)
    qkv_w = np.asarray(inputs["qkv_w"], np.float32)
    proj_w = np.asarray(inputs["proj_w"], np.float32)
    proj_b = np.asarray(inputs["proj_b"], np.float32)

    nc = _get_program()
    res = run_bass_kernel_spmd(nc, _in_maps(x, qkv_w, proj_w),
                               core_ids=list(range(N_CORES)))
    out = np.empty((B, N, C), np.float32)
    for c in range(N_CORES):
        b, p = divmod(c, 4)
        out[b, p * 512:(p + 1) * 512, :] = res.results[c]["part"]
    out += proj_b
    return out
